# revision 75
# baseline (speedup 1.0000x reference)
"""Trainium2 Bass kernel: masked multi-head attention, sharded across 8 NeuronCores.

Problem shapes (hardcoded): B=2, T=2048, D=1024, H=16 heads, dh=64.

Sharding: one SPMD program with two phases (one per batch element). In each
phase every core handles 2 of the 16 heads (core c -> heads 2c, 2c+1), so the
16 heads of each batch are spread over all 8 cores. This load-balances the
data-dependent work (Q_len/V_len trim the q/k tile counts per batch).

All matmul operands are fp16 (inputs cast on host): fp32 matmuls cost 4
cycles/row on the TRN2 PE vs 1 for fp16, and fp16 halves the input DMA bytes.
PSUM accumulation stays fp32, so the error vs the fp32 reference is ~1e-3.

Schedule: the first phase's attention is split into kv-chunk PASSES (kt 0..3,
then kt 4..NK-1), each accumulating a partial softmax numerator + denominator
(host sums the partials), so attention starts after only the first 512-key
chunk of K/V lands. Attention is emitted as a FLAT pipeline of (q-chunk,
kt-pair) units -- S matmuls + one exp per head per unit, skewed one unit
ahead of the PV matmuls -- and every later projection (q chunks 1.., later
k/v chunks, and the whole second phase's projections) is chopped into
single-matmul FILLER thunks drained between units: exp on the ACT engine
paces the units (~2us per unit vs ~1.1us of PE work), and the fillers soak
up the PE slack. PSUM: 3 "sp" slots of 2 banks (S pairs + one open filler
projection accumulator) + 2 "ot" banks (PV accumulators). A dummy matmul
stream at t~7us releases the HAM clock throttle before the real work.
Output DMAs are deferred behind later input dma_starts (in-order sync queue).
v_aug rows for tokens >= V_len are zero (host zeroes V) and their ones-column
entry is zeroed on device, replacing the additive -1e12 key mask exactly.
The host does the final divide-by-denominator, query mask and transpose.
"""

import math
import os
from contextlib import ExitStack

import numpy as np

import concourse.bacc as bacc
import concourse.mybir as mybir
import concourse.tile as tile
from concourse.bass_utils import run_bass_kernel_spmd

F32 = mybir.dt.float32
F16 = mybir.dt.float16
EXP = mybir.ActivationFunctionType.Exp
XNP = np.float16

B, T, D, H, DH = 2, 2048, 1024, 16, 64
N_CORES = 8
KCH = D // 128          # 8 contraction chunks of the model dim
SCALE = 1.0 / math.sqrt(DH)
FILL_PER_UNIT = 5       # filler thunks drained after each attention unit

LAST_EXEC_NS = None     # filled when BASS_TRACE=1


def _ensure_ntff_hook():
    """run_bass_kernel_spmd(trace=True) imports antenv.axon_hooks, which some
    containers lack; synthesize it (backed by libaxon_pjrt's NRT profiling)
    so tracing degrades gracefully instead of crashing."""
    import sys
    import types
    try:
        import antenv.axon_hooks  # noqa: F401
        return
    except ImportError:
        pass
    try:
        import antenv
        from trn_agent_boot.trn_boot import _ntff_profile_via_ctypes
        hook = _ntff_profile_via_ctypes("/opt/axon/libaxon_pjrt.so")
    except Exception:
        antenv = None
        hook = None
    try:
        m = types.ModuleType("antenv.axon_hooks")
        m._hook = hook
        m.set_axon_ntff_profile_hook = lambda h: setattr(m, "_hook", h)
        m.get_axon_ntff_profile_hook = lambda: m._hook
        sys.modules["antenv.axon_hooks"] = m
        if antenv is not None:
            antenv.axon_hooks = m
    except Exception:
        pass


def _ceil_div(a, b):
    return -(-a // b)


def _chunk_widths(Qp, first=False):
    """Balanced q-chunk widths (multiples of 32, <=512) -- avoids a
    degenerate tail chunk whose tiny matmuls/exps are all overhead. The
    first phase gets a narrow 256-wide first chunk so attention starts on
    less lead-in DMA."""
    NQC = _ceil_div(Qp, 512)
    base = (Qp // NQC) // 32 * 32
    rem = (Qp - base * NQC) // 32
    return [base + 32 * (i < rem) for i in range(NQC)]


class _FillQueue:
    """Deadline-ordered queue of single-instruction emitter thunks."""

    def __init__(self):
        self.q = []
        self.cnt = {}

    def add(self, tag, thunks):
        for t in thunks:
            self.q.append((tag, t))
        self.cnt[tag] = self.cnt.get(tag, 0) + len(thunks)

    def pop(self, k):
        while k > 0 and self.q:
            tag, t = self.q.pop(0)
            self.cnt[tag] -= 1
            t()
            k -= 1

    def drain(self, tag):
        while self.cnt.get(tag, 0) > 0:
            tg, t = self.q.pop(0)
            self.cnt[tg] -= 1
            t()


def _emit_all(nc, tc, P, phases):
    wts = P["wts"]
    fq = _FillQueue()
    deferred = []   # out-DMA thunks, flushed behind later input dma_starts

    for ph in phases:
        s = str(ph["b"])
        ph["ws"] = _chunk_widths(ph["Qp"], ph.get("first"))
        ph["cst"] = [sum(ph["ws"][:i]) for i in range(len(ph["ws"]))]
        Kp = ph["Kp"]
        kw = []
        r = Kp
        while r > 0:
            kw.append(min(512, r))
            r -= kw[-1]
        ph["kw"] = kw
        ph["kst"] = [sum(kw[:i]) for i in range(len(kw))]
        ph["kmap"] = []
        for ci, w in enumerate(kw):
            for off in range(w // 128):
                ph["kmap"].append((ci, off))
        ph["kcs"] = [None] * len(kw)
        ph["vas"] = [None] * ph["NK"]
        ph["qcs"] = {}
        kts_all = list(range(ph["NK"]))
        na = kw[0] // 128
        ph["passes"] = ([kts_all[:na], kts_all[na:]]
                        if ph.get("first") and ph["NK"] > na else [kts_all])
        ph["npass"] = len(ph["passes"])

    def kproj_thunks(ph, c):
        """DMA issues now; returns single-matmul thunks + the kT copy."""
        s, io = str(ph["b"]), ph["io"]
        n, k0 = ph["kw"][c], ph["kst"][c]
        xt = P["x"].tile([128, KCH, n], F16, tag="xt", name="xt", bufs=3)
        kc = P["persist"].tile([128, n], F16, tag="kT" + s, name="kT" + s,
                               bufs=len(ph["kcs"]))
        ph["kcs"][c] = kc
        box = {}   # psum tile allocated at emission position, not build time
        if ph.get("first") and c == 0:
            # quarters: one dma_start's descriptors fan out over only a few
            # DMA queues (~55GB/s), so parallel splits speed up the lead-in
            for q in range(4):
                nc.sync.dma_start(xt[:, 2 * q:2 * q + 2, :],
                                  io["xk"][:, 2 * q:2 * q + 2, 0:n])
        else:
            # one dma_start per chunk elsewhere: each costs ~600ns of
            # serialized sync-sequencer issue + ~1us of descriptor gen
            nc.sync.dma_start(xt[:], io["xk"][:, :, k0:k0 + n])

        def mk(k):
            def go():
                if k == 0:
                    box["ps"] = P["sp"].tile([128, n], F32, tag="sp", name="kps")
                nc.tensor.matmul(
                    box["ps"][:], lhsT=wts["wk"][:, k, :], rhs=xt[:, k, :],
                    start=(k == 0), stop=(k == KCH - 1), skip_group_check=True)
            return go
        return [mk(k) for k in range(KCH)] + [
            lambda: nc.vector.tensor_copy(kc[:], box["ps"][:])]

    def vproj_thunks(ph, c):
        s, io = str(ph["b"]), ph["io"]
        NK, vrem = ph["NK"], ph["vrem"]
        n, k0 = ph["kw"][c], ph["kst"][c]
        ng = n // 128
        xtv = P["x"].tile([128, KCH, n], F16, tag="xtv", name="xtv", bufs=3)
        if ph.get("first") and c == 0:
            # lead-in: one half on the (idle) ACT engine's DGE in parallel
            # with the sync queue's xk quarters, the other behind xk on sync
            nc.scalar.dma_start(xtv[:, 0:4, :], io["xv"][:, 0:4, 0:n])
            nc.sync.dma_start(xtv[:, 4:8, :], io["xv"][:, 4:8, 0:n])
        else:
            nc.sync.dma_start(xtv[:], io["xv"][:, :, k0:k0 + n])
        thunks = []
        for j in range(ng):
            kt = k0 // 128 + j
            # 128 weight columns (65 used) so the PV matmul's LDWEIGHTS gets
            # fast-weight-load and hides under the previous matmul
            va = P["persist"].tile([128, 2, 128], F16, tag="va" + s,
                                   name="va" + s, bufs=NK)
            if kt == NK - 1 and vrem is not None:
                # partial last key tile: ones only on the valid rows, so
                # padded keys add nothing to the softmax denominator
                nc.vector.memset(va[:, :, 64:128], 0.0)
                nc.vector.memset(va[0:vrem, :, 64:65], 1.0)
            else:
                nc.vector.memset(va[:, :, 64:128], 1.0)
            ph["vas"][kt] = va
            box = {}

            def mk(k, j=j, box=box):
                def go():
                    if k == 0:
                        box["ps"] = P["sp"].tile([128, 128], F32, tag="sp",
                                                 name="ps2")
                    nc.tensor.matmul(
                        box["ps"][:], lhsT=xtv[:, k, j * 128:(j + 1) * 128],
                        rhs=wts["wv"][:, k, :],
                        start=(k == 0), stop=(k == KCH - 1),
                        skip_group_check=True)
                return go
            thunks += [mk(k) for k in range(KCH)]
            thunks.append(lambda va=va, box=box: nc.vector.tensor_copy(
                va[:, :, 0:64], box["ps"][:].rearrange("p (g d) -> p g d", g=2)))
        return thunks

    def qproj_thunks(ph, c):
        s, io = str(ph["b"]), ph["io"]
        n, c0 = ph["ws"][c], ph["cst"][c]
        xtq = P["x"].tile([128, KCH, n], F16, tag="xtq", name="xtq", bufs=4)
        if ph.get("first") and c == 0:
            nc.scalar.dma_start(xtq[:, 0:4, :], io["xq"][:, 0:4, c0:c0 + n])
            nc.sync.dma_start(xtq[:, 4:8, :], io["xq"][:, 4:8, c0:c0 + n])
        else:
            nc.sync.dma_start(xtq[:], io["xq"][:, :, c0:c0 + n])
        qc = P["persist"].tile([128, n], F16, tag="qT" + s, name="qT" + s,
                               bufs=len(ph["ws"]))
        ph["qcs"][c] = qc
        box = {}

        def mk(k):
            def go():
                if k == 0:
                    box["ps"] = P["sp"].tile([128, n], F32, tag="sp", name="qps")
                nc.tensor.matmul(
                    box["ps"][:], lhsT=wts["wq"][:, k, :], rhs=xtq[:, k, :],
                    start=(k == 0), stop=(k == KCH - 1), skip_group_check=True)
            return go
        return [mk(k) for k in range(KCH)] + [
            lambda: nc.vector.tensor_copy(qc[:], box["ps"][:])]

    def run_pass(ph, kts, out_d, pi):
        """Flat skew-1 pipeline over (chunk, kt-pair) units with fillers."""
        s, scale = str(ph["b"]), ph["scale"]
        # one SBUF staging buffer for the whole pass -> a single out-DMA
        obp = P["ob"].tile([65, 2, ph["Qp"]], F32, tag=f"ob{s}_{pi}",
                           name="obp", bufs=1)
        kcs, vas, qcs = ph["kcs"], ph["vas"], ph["qcs"]
        units = []
        groups = [kts[j:j + 2] for j in range(0, len(kts), 2)]
        for c in range(len(ph["ws"])):
            for gi, g in enumerate(groups):
                units.append((c, g, gi == 0, gi == len(groups) - 1))
        otds = {}

        def emit_sg(u):
            c, g, first, last = u
            n = ph["ws"][c]
            qc = qcs[c]
            es = []
            for h in (0, 1):
                # pad the pair stride to a full 2KB PSUM bank: a matmul
                # output slice must not straddle a bank boundary
                sps = P["sp"].tile([128, len(g), n], F32, tag="sp", name="sps",
                                   padded_shape=[128, len(g), 512])
                for i, kt in enumerate(g):
                    ci, off = ph["kmap"][kt]
                    nc.tensor.matmul(
                        sps[:, i, :],
                        lhsT=kcs[ci][h * 64:(h + 1) * 64,
                                     off * 128:off * 128 + 128],
                        rhs=qc[h * 64:(h + 1) * 64, :],
                        start=True, stop=True)
                e = P["e"].tile([128, len(g), n], F16, tag="e", name="e")
                nc.scalar.activation(e[:], sps[:], EXP, scale=scale)
                es.append(e)
            return es

        def emit_pv(u, es):
            c, g, first, last = u
            n, c0 = ph["ws"][c], ph["cst"][c]
            if first:
                otds[c] = [P["ot"].tile([128, n], F32, tag="ot", name="otd")
                           for _ in (0, 1)]
            for h in (0, 1):
                for i, kt in enumerate(g):
                    nc.tensor.matmul(otds[c][h][:], lhsT=vas[kt][:, h, :],
                                     rhs=es[h][:, i, :],
                                     start=(kt == kts[0]), stop=(kt == kts[-1]),
                                     skip_group_check=True)
            if last:
                for h in (0, 1):
                    nc.vector.tensor_copy(obp[:, h, c0:c0 + n],
                                          otds[c][h][0:65, :])

        prev = None
        for u in units:
            if u[2]:  # first unit of chunk c: its qc must be projected
                fq.drain(("q", s, u[0]))
            for kt in u[1]:
                fq.drain(("kv", s, ph["kmap"][kt][0]))
            es = emit_sg(u)
            if prev is not None:
                emit_pv(*prev)
            prev = (u, es)
            fq.pop(FILL_PER_UNIT)
        emit_pv(*prev)
        deferred.append(lambda: nc.sync.dma_start(out_d[:], obp[:]))

    # ---- phase 0 lead-in: first kv chunk + first q chunk, emitted directly
    # with dummy-matmul bursts plugging the DMA-paced PE waits
    ph0 = phases[0]
    kth = kproj_thunks(ph0, 0)
    vth = vproj_thunks(ph0, 0)
    qth = qproj_thunks(ph0, 0)
    for t in kth[:4]:
        t()
    P["wfill"](6)
    for t in kth[4:]:
        t()
    P["wfill"](6)
    for t in vth:
        t()
    P["wfill"](6)
    for t in qth:
        t()
    nxt = phases[1] if len(phases) > 1 else None

    for pi, kts in enumerate(ph0["passes"]):
        last_pass = pi == len(ph0["passes"]) - 1
        s0 = str(ph0["b"])
        if pi == 0:
            # queue later q chunks (deadline: their attention chunk), later
            # kv chunks (deadline: pass B), and the whole second phase's
            # projections (deadline: its attention) as fillers -- all their
            # input DMAs issue NOW, long before the matmuls need them
            for c in range(1, len(ph0["ws"])):
                fq.add(("q", s0, c), qproj_thunks(ph0, c))
            for c in range(1, len(ph0["kcs"])):
                fq.add(("kv", s0, c),
                       kproj_thunks(ph0, c) + vproj_thunks(ph0, c))
            if nxt is not None:
                s1 = str(nxt["b"])
                for c in range(len(nxt["kcs"])):
                    fq.add(("kv", s1, c),
                           kproj_thunks(nxt, c) + vproj_thunks(nxt, c))
                for c in range(len(nxt["ws"])):
                    fq.add(("q", s1, c), qproj_thunks(nxt, c))
        else:
            for t in deferred:
                t()
            deferred.clear()
        run_pass(ph0, kts, ph0["io"]["out"][pi], pi)

    if nxt is not None:
        for t in deferred:
            t()
        deferred.clear()
        for kts in nxt["passes"]:
            run_pass(nxt, kts, nxt["io"]["out"][0], 0)
    for t in deferred:
        t()


def _build_program(phases):
    nc = bacc.Bacc("TRN2", target_bir_lowering=False, debug=False,
                   num_devices=N_CORES)
    for ph in phases:
        s = str(ph["b"])
        Qp, Kp, NK = ph["Qp"], ph["Kp"], ph["NK"]
        npass = 2 if (ph.get("first") and NK > 4) else 1
        io = {
            "xq": nc.dram_tensor("xq" + s, [128, KCH, Qp], F16, kind="ExternalInput"),
            "xk": nc.dram_tensor("xk" + s, [128, KCH, Kp], F16, kind="ExternalInput"),
            "xv": nc.dram_tensor("xv" + s, [128, KCH, Kp], F16, kind="ExternalInput"),
            "out": [nc.dram_tensor(f"out{s}_{p}", [65, 2, Qp], F32,
                                   kind="ExternalOutput") for p in range(npass)],
        }
        ph["io"] = io

    with tile.TileContext(nc) as tc, ExitStack() as ctx:
        P = {
            "w": ctx.enter_context(tc.tile_pool(name="w", bufs=1)),
            "x": ctx.enter_context(tc.tile_pool(name="x", bufs=3)),
            "e": ctx.enter_context(tc.tile_pool(name="e", bufs=6)),
            "ob": ctx.enter_context(tc.tile_pool(name="ob", bufs=10)),
            "persist": ctx.enter_context(tc.tile_pool(name="persist", bufs=1)),
            # 3 sp slots of 2 PSUM banks: two in-flight S pairs + one open
            # filler projection accumulator; ot = 2 banks of PV accumulators
            "sp": ctx.enter_context(tc.tile_pool(name="sp", bufs=3, space="PSUM")),
            "ot": ctx.enter_context(tc.tile_pool(name="ot", bufs=2, space="PSUM")),
        }
        # prime the ACT exp table while the first DMAs are in flight
        warm = P["w"].tile([1, 1], F32, tag="actwarm", name="actwarm")
        nc.vector.memset(warm[:], 0.0)
        nc.scalar.activation(warm[:], warm[:], EXP)
        # dummy matmul stream: ~4us of sustained PE activity releases the
        # HAM clock throttle (1.2 -> 2.4 GHz) while the first input DMAs are
        # still in flight, so the real matmuls start warm. wfill() re-issues
        # short bursts into the (lead-in-idle) ot banks to plug DMA waits.
        wu = P["w"].tile([128, 128], F16, tag="wu", name="wu")
        nc.vector.memset(wu[:], 0.0)
        P["wu"] = wu

        def wfill(n):
            wups = P["ot"].tile([128, 128], F32, tag="ot", name="wups")
            for _ in range(n):
                nc.tensor.matmul(wups[:], lhsT=wu[:], rhs=wu[:],
                                 start=True, stop=True, skip_group_check=True)
        P["wfill"] = wfill
        wfill(44)
        # all three weights in one packed DMA, issued from the (idle) ACT
        # engine so it overlaps the sync queue's first xk issue
        wpd = nc.dram_tensor("wpk", [128, 3, KCH, 128], F16, kind="ExternalInput")
        wpt = P["w"].tile([128, 3, KCH, 128], F16, tag="wpk", name="wpk")
        nc.scalar.dma_start(wpt[:], wpd[:])
        P["wts"] = {"wk": wpt[:, 0], "wv": wpt[:, 1], "wq": wpt[:, 2]}
        _emit_all(nc, tc, P, phases)
    nc.compile()
    return nc


def _prep_xT(X, P):
    """[T, D] -> [128, KCH, P] with x[p, k, t] = X[t, k*128 + p]."""
    Xp = np.ascontiguousarray(X[:P].T)                 # [D, P]
    return np.ascontiguousarray(
        Xp.reshape(KCH, 128, P).transpose(1, 0, 2)).astype(XNP)  # [128, KCH, P]


def _prep_w(W, c):
    """[D, H*DH] -> per-core [128, KCH, 128] slice of heads (2c, 2c+1)."""
    Ws = W[:, c * 128:(c + 1) * 128]                   # [D, 128]
    return np.ascontiguousarray(
        Ws.reshape(KCH, 128, 128).transpose(1, 0, 2)).astype(XNP)


def kernel(Q_seq, K_seq, V_seq, Q_len, V_len, WQ, WK, WV):
    global LAST_EXEC_NS
    Q_seq = np.asarray(Q_seq, dtype=np.float32)
    K_seq = np.asarray(K_seq, dtype=np.float32)
    V_seq = np.asarray(V_seq, dtype=np.float32)
    WQ = np.asarray(WQ, dtype=np.float32)
    WK = np.asarray(WK, dtype=np.float32)
    WV = np.asarray(WV, dtype=np.float32)
    qlen = [int(np.asarray(Q_len)[b, 0]) for b in range(B)]
    vlen = [int(np.asarray(V_len)[b, 0]) for b in range(B)]

    phases = []
    for b in range(B):
        Qp = _ceil_div(qlen[b], 32) * 32   # q only needs 32-elem alignment
        if Qp == 0:
            continue  # whole batch output is zero
        if vlen[b] > 0:
            NK, scale = _ceil_div(vlen[b], 128), SCALE
            vrem = vlen[b] - (NK - 1) * 128
            if vrem == 128:
                vrem = None
        else:
            # all keys masked -> reference softmax degenerates to uniform
            # over all T keys; exp(0*S) = 1 reproduces it exactly.
            NK, scale, vrem = T // 128, 0.0, None
        phases.append(dict(b=b, NK=NK, Qp=Qp, Kp=NK * 128, scale=scale,
                           vrem=vrem, first=not phases))

    out = np.zeros((B, T, H * DH), dtype=np.float32)
    if not phases:
        return out

    nc = _build_program(phases)

    # per-phase data shared by all cores
    shared = {}
    for ph in phases:
        b, s, Kp = ph["b"], str(ph["b"]), ph["Kp"]
        Vb = V_seq[b]
        if 0 < vlen[b] < Kp:
            Vb = Vb.copy()
            Vb[vlen[b]:Kp] = 0.0   # padded keys: zero v rows -> no output term
        shared[s] = {
            "xq" + s: _prep_xT(Q_seq[b], ph["Qp"]),
            "xk" + s: _prep_xT(K_seq[b], Kp),
            "xv" + s: _prep_xT(Vb, Kp),
        }

    in_maps = []
    for c in range(N_CORES):
        m = {}
        for ph in phases:
            m.update(shared[str(ph["b"])])
        m["wpk"] = np.ascontiguousarray(np.stack(
            [_prep_w(WK, c), _prep_w(WV, c), _prep_w(WQ, c)], axis=1))
        in_maps.append(m)

    trace = bool(os.environ.get("BASS_TRACE"))
    if trace:
        _ensure_ntff_hook()
    res = run_bass_kernel_spmd(nc, in_maps, list(range(N_CORES)), trace=trace)
    LAST_EXEC_NS = res.exec_time_ns

    for c in range(N_CORES):
        r = res.results[c]
        for ph in phases:
            b, s = ph["b"], str(ph["b"])
            ql = qlen[b]
            acc = r["out" + s + "_0"].astype(np.float64)
            for p in range(1, ph["npass"]):
                acc += r[f"out{s}_{p}"]
            for h in (0, 1):
                head = 2 * c + h
                num = acc[0:64, h, :ql]
                den = acc[64, h, :ql]
                out[b, :ql, head * DH:(head + 1) * DH] = (num / den).T
    return out


# revision 76
# speedup vs baseline: 1.0145x; 1.0145x over previous
"""Trainium2 Bass kernel: masked multi-head attention, sharded across 8 NeuronCores.

Problem shapes (hardcoded): B=2, T=2048, D=1024, H=16 heads, dh=64.

Sharding: one SPMD program with two phases (one per batch element). In each
phase every core handles 2 of the 16 heads (core c -> heads 2c, 2c+1), so the
16 heads of each batch are spread over all 8 cores. This load-balances the
data-dependent work (Q_len/V_len trim the q/k tile counts per batch).

All matmul operands are fp16 (inputs cast on host): fp32 matmuls cost 4
cycles/row on the TRN2 PE vs 1 for fp16, and fp16 halves the input DMA bytes.
PSUM accumulation stays fp32, so the error vs the fp32 reference is ~1e-3.

Schedule: the first phase's attention is split into kv-chunk PASSES (kt 0..3,
then kt 4..NK-1), each accumulating a partial softmax numerator + denominator
(host sums the partials), so attention starts after only the first 512-key
chunk of K/V lands. Attention is emitted as a FLAT pipeline of (q-chunk,
kt-pair) units -- S matmuls + one exp per head per unit, skewed one unit
ahead of the PV matmuls -- and every later projection (q chunks 1.., later
k/v chunks, and the whole second phase's projections) is chopped into
single-matmul FILLER thunks drained between units: exp on the ACT engine
paces the units (~2us per unit vs ~1.1us of PE work), and the fillers soak
up the PE slack. PSUM: 3 "sp" slots of 2 banks (S pairs + one open filler
projection accumulator) + 2 "ot" banks (PV accumulators). A dummy matmul
stream at t~7us releases the HAM clock throttle before the real work.
Output DMAs are deferred behind later input dma_starts (in-order sync queue).
v_aug rows for tokens >= V_len are zero (host zeroes V) and their ones-column
entry is zeroed on device, replacing the additive -1e12 key mask exactly.
The host does the final divide-by-denominator, query mask and transpose.
"""

import math
import os
from contextlib import ExitStack

import numpy as np

import concourse.bacc as bacc
import concourse.mybir as mybir
import concourse.tile as tile
from concourse.bass_utils import run_bass_kernel_spmd

F32 = mybir.dt.float32
F16 = mybir.dt.float16
EXP = mybir.ActivationFunctionType.Exp
XNP = np.float16

B, T, D, H, DH = 2, 2048, 1024, 16, 64
N_CORES = 8
KCH = D // 128          # 8 contraction chunks of the model dim
SCALE = 1.0 / math.sqrt(DH)
FILL_PER_UNIT = 5       # filler thunks drained after each attention unit

LAST_EXEC_NS = None     # filled when BASS_TRACE=1


def _ensure_ntff_hook():
    """run_bass_kernel_spmd(trace=True) imports antenv.axon_hooks, which some
    containers lack; synthesize it (backed by libaxon_pjrt's NRT profiling)
    so tracing degrades gracefully instead of crashing."""
    import sys
    import types
    try:
        import antenv.axon_hooks  # noqa: F401
        return
    except ImportError:
        pass
    try:
        import antenv
        from trn_agent_boot.trn_boot import _ntff_profile_via_ctypes
        hook = _ntff_profile_via_ctypes("/opt/axon/libaxon_pjrt.so")
    except Exception:
        antenv = None
        hook = None
    try:
        m = types.ModuleType("antenv.axon_hooks")
        m._hook = hook
        m.set_axon_ntff_profile_hook = lambda h: setattr(m, "_hook", h)
        m.get_axon_ntff_profile_hook = lambda: m._hook
        sys.modules["antenv.axon_hooks"] = m
        if antenv is not None:
            antenv.axon_hooks = m
    except Exception:
        pass


def _ceil_div(a, b):
    return -(-a // b)


def _chunk_widths(Qp, first=False):
    """Balanced q-chunk widths (multiples of 32, <=512) -- avoids a
    degenerate tail chunk whose tiny matmuls/exps are all overhead. The
    first phase gets a narrow 256-wide first chunk so attention starts on
    less lead-in DMA."""
    NQC = _ceil_div(Qp, 512)
    base = (Qp // NQC) // 32 * 32
    rem = (Qp - base * NQC) // 32
    return [base + 32 * (i < rem) for i in range(NQC)]


class _FillQueue:
    """Deadline-ordered queue of single-instruction emitter thunks."""

    def __init__(self):
        self.q = []
        self.cnt = {}

    def add(self, tag, thunks):
        for t in thunks:
            self.q.append((tag, t))
        self.cnt[tag] = self.cnt.get(tag, 0) + len(thunks)

    def pop(self, k):
        while k > 0 and self.q:
            tag, t = self.q.pop(0)
            self.cnt[tag] -= 1
            t()
            k -= 1

    def drain(self, tag):
        while self.cnt.get(tag, 0) > 0:
            tg, t = self.q.pop(0)
            self.cnt[tg] -= 1
            t()


def _emit_all(nc, tc, P, phases):
    wts = P["wts"]
    fq = _FillQueue()
    deferred = []   # out-DMA thunks, flushed behind later input dma_starts

    for ph in phases:
        s = str(ph["b"])
        ph["ws"] = _chunk_widths(ph["Qp"], ph.get("first"))
        ph["cst"] = [sum(ph["ws"][:i]) for i in range(len(ph["ws"]))]
        Kp = ph["Kp"]
        kw = []
        r = Kp
        while r > 0:
            kw.append(min(512, r))
            r -= kw[-1]
        ph["kw"] = kw
        ph["kst"] = [sum(kw[:i]) for i in range(len(kw))]
        ph["kmap"] = []
        for ci, w in enumerate(kw):
            for off in range(w // 128):
                ph["kmap"].append((ci, off))
        ph["kcs"] = [None] * len(kw)
        ph["vas"] = [None] * ph["NK"]
        ph["qcs"] = {}
        kts_all = list(range(ph["NK"]))
        na = kw[0] // 128
        ph["passes"] = ([kts_all[:na], kts_all[na:]]
                        if ph.get("first") and ph["NK"] > na else [kts_all])
        ph["npass"] = len(ph["passes"])

    def kproj_thunks(ph, c):
        """DMA issues now; returns single-matmul thunks + the kT copy."""
        s, io = str(ph["b"]), ph["io"]
        n, k0 = ph["kw"][c], ph["kst"][c]
        xt = P["x"].tile([128, KCH, n], F16, tag="xt", name="xt", bufs=4)
        kc = P["persist"].tile([128, n], F16, tag="kT" + s, name="kT" + s,
                               bufs=len(ph["kcs"]))
        ph["kcs"][c] = kc
        box = {}   # psum tile allocated at emission position, not build time
        if ph.get("first") and c == 0:
            # quarters: one dma_start's descriptors fan out over only a few
            # DMA queues (~55GB/s), so parallel splits speed up the lead-in
            for q in range(4):
                nc.sync.dma_start(xt[:, 2 * q:2 * q + 2, :],
                                  io["xk"][:, 2 * q:2 * q + 2, 0:n])
        else:
            # one dma_start per chunk elsewhere: each costs ~600ns of
            # serialized sync-sequencer issue + ~1us of descriptor gen
            nc.sync.dma_start(xt[:], io["xk"][:, :, k0:k0 + n])

        def mk(k):
            def go():
                if k == 0:
                    box["ps"] = P["sp"].tile([128, n], F32, tag="sp", name="kps")
                nc.tensor.matmul(
                    box["ps"][:], lhsT=wts["wk"][:, k, :], rhs=xt[:, k, :],
                    start=(k == 0), stop=(k == KCH - 1), skip_group_check=True)
            return go
        return [mk(k) for k in range(KCH)] + [
            lambda: nc.vector.tensor_copy(kc[:], box["ps"][:])]

    def vproj_thunks(ph, c):
        s, io = str(ph["b"]), ph["io"]
        NK, vrem = ph["NK"], ph["vrem"]
        n, k0 = ph["kw"][c], ph["kst"][c]
        ng = n // 128
        xtv = P["x"].tile([128, KCH, n], F16, tag="xtv", name="xtv", bufs=4)
        if ph.get("first") and c == 0:
            # lead-in: one half on the (idle) ACT engine's DGE in parallel
            # with the sync queue's xk quarters, the other behind xk on sync
            nc.scalar.dma_start(xtv[:, 0:4, :], io["xv"][:, 0:4, 0:n])
            nc.sync.dma_start(xtv[:, 4:8, :], io["xv"][:, 4:8, 0:n])
        else:
            nc.sync.dma_start(xtv[:], io["xv"][:, :, k0:k0 + n])
        thunks = []
        for j in range(ng):
            kt = k0 // 128 + j
            # 128 weight columns (65 used) so the PV matmul's LDWEIGHTS gets
            # fast-weight-load and hides under the previous matmul
            va = P["persist"].tile([128, 2, 128], F16, tag="va" + s,
                                   name="va" + s, bufs=NK)
            if kt == NK - 1 and vrem is not None:
                # partial last key tile: ones only on the valid rows, so
                # padded keys add nothing to the softmax denominator
                nc.vector.memset(va[:, :, 64:128], 0.0)
                nc.vector.memset(va[0:vrem, :, 64:65], 1.0)
            else:
                nc.vector.memset(va[:, :, 64:128], 1.0)
            ph["vas"][kt] = va
            box = {}

            def mk(k, j=j, box=box):
                def go():
                    if k == 0:
                        box["ps"] = P["sp"].tile([128, 128], F32, tag="sp",
                                                 name="ps2")
                    nc.tensor.matmul(
                        box["ps"][:], lhsT=xtv[:, k, j * 128:(j + 1) * 128],
                        rhs=wts["wv"][:, k, :],
                        start=(k == 0), stop=(k == KCH - 1),
                        skip_group_check=True)
                return go
            thunks += [mk(k) for k in range(KCH)]
            thunks.append(lambda va=va, box=box: nc.vector.tensor_copy(
                va[:, :, 0:64], box["ps"][:].rearrange("p (g d) -> p g d", g=2)))
        return thunks

    def qproj_thunks(ph, c):
        s, io = str(ph["b"]), ph["io"]
        n, c0 = ph["ws"][c], ph["cst"][c]
        xtq = P["x"].tile([128, KCH, n], F16, tag="xtq", name="xtq", bufs=4)
        if ph.get("first") and c == 0:
            nc.scalar.dma_start(xtq[:, 0:4, :], io["xq"][:, 0:4, c0:c0 + n])
            nc.sync.dma_start(xtq[:, 4:8, :], io["xq"][:, 4:8, c0:c0 + n])
        else:
            nc.sync.dma_start(xtq[:], io["xq"][:, :, c0:c0 + n])
        qc = P["persist"].tile([128, n], F16, tag="qT" + s, name="qT" + s,
                               bufs=len(ph["ws"]))
        ph["qcs"][c] = qc
        box = {}

        def mk(k):
            def go():
                if k == 0:
                    box["ps"] = P["sp"].tile([128, n], F32, tag="sp", name="qps")
                nc.tensor.matmul(
                    box["ps"][:], lhsT=wts["wq"][:, k, :], rhs=xtq[:, k, :],
                    start=(k == 0), stop=(k == KCH - 1), skip_group_check=True)
            return go
        return [mk(k) for k in range(KCH)] + [
            lambda: nc.vector.tensor_copy(qc[:], box["ps"][:])]

    def run_pass(ph, kts, out_d, pi):
        """Flat skew-1 pipeline over (chunk, kt-pair) units with fillers."""
        s, scale = str(ph["b"]), ph["scale"]
        # one SBUF staging buffer for the whole pass -> a single out-DMA
        obp = P["ob"].tile([65, 2, ph["Qp"]], F32, tag=f"ob{s}_{pi}",
                           name="obp", bufs=1)
        kcs, vas, qcs = ph["kcs"], ph["vas"], ph["qcs"]
        units = []
        groups = [kts[j:j + 2] for j in range(0, len(kts), 2)]
        for c in range(len(ph["ws"])):
            for gi, g in enumerate(groups):
                units.append((c, g, gi == 0, gi == len(groups) - 1))
        otds = {}

        def emit_sg(u):
            c, g, first, last = u
            n = ph["ws"][c]
            qc = qcs[c]
            es = []
            for h in (0, 1):
                # pad the pair stride to a full 2KB PSUM bank: a matmul
                # output slice must not straddle a bank boundary
                sps = P["sp"].tile([128, len(g), n], F32, tag="sp", name="sps",
                                   padded_shape=[128, len(g), 512])
                for i, kt in enumerate(g):
                    ci, off = ph["kmap"][kt]
                    nc.tensor.matmul(
                        sps[:, i, :],
                        lhsT=kcs[ci][h * 64:(h + 1) * 64,
                                     off * 128:off * 128 + 128],
                        rhs=qc[h * 64:(h + 1) * 64, :],
                        start=True, stop=True)
                e = P["e"].tile([128, len(g), n], F16, tag="e", name="e")
                nc.scalar.activation(e[:], sps[:], EXP, scale=scale)
                es.append(e)
            return es

        def emit_pv(u, es):
            c, g, first, last = u
            n, c0 = ph["ws"][c], ph["cst"][c]
            if first:
                otds[c] = [P["ot"].tile([128, n], F32, tag="ot", name="otd")
                           for _ in (0, 1)]
            for h in (0, 1):
                for i, kt in enumerate(g):
                    nc.tensor.matmul(otds[c][h][:], lhsT=vas[kt][:, h, :],
                                     rhs=es[h][:, i, :],
                                     start=(kt == kts[0]), stop=(kt == kts[-1]),
                                     skip_group_check=True)
            if last:
                for h in (0, 1):
                    nc.vector.tensor_copy(obp[:, h, c0:c0 + n],
                                          otds[c][h][0:65, :])

        prev = None
        for u in units:
            if u[2]:  # first unit of chunk c: its qc must be projected
                fq.drain(("q", s, u[0]))
            es = emit_sg(u)
            if prev is not None:
                emit_pv(*prev)
            prev = (u, es)
            fq.pop(FILL_PER_UNIT)
        emit_pv(*prev)
        deferred.append(lambda: nc.sync.dma_start(out_d[:], obp[:]))

    # ---- phase 0 lead-in: first kv chunk + first q chunk, emitted directly
    # with dummy-matmul bursts plugging the DMA-paced PE waits
    ph0 = phases[0]
    kth = kproj_thunks(ph0, 0)
    vth = vproj_thunks(ph0, 0)
    qth = qproj_thunks(ph0, 0)
    for t in kth[:4]:
        t()
    P["wfill"](6)
    for t in kth[4:]:
        t()
    P["wfill"](6)
    for t in vth:
        t()
    P["wfill"](6)
    for t in qth:
        t()
    nxt = phases[1] if len(phases) > 1 else None

    for pi, kts in enumerate(ph0["passes"]):
        last_pass = pi == len(ph0["passes"]) - 1
        s0 = str(ph0["b"])
        if pi == 0:
            # queue later q chunks (deadline: their attention chunk), later
            # kv chunks (deadline: pass B), and the whole second phase's
            # projections (deadline: its attention) as fillers -- all their
            # input DMAs issue NOW, long before the matmuls need them
            for c in range(1, len(ph0["ws"])):
                fq.add(("q", s0, c), qproj_thunks(ph0, c))
            for c in range(1, len(ph0["kcs"])):
                fq.add(("kv", s0), kproj_thunks(ph0, c) + vproj_thunks(ph0, c))
            if nxt is not None:
                s1 = str(nxt["b"])
                for c in range(len(nxt["kcs"])):
                    fq.add(("kv", s1),
                           kproj_thunks(nxt, c) + vproj_thunks(nxt, c))
                for c in range(len(nxt["ws"])):
                    fq.add(("q", s1, c), qproj_thunks(nxt, c))
        else:
            fq.drain(("kv", s0))
            for t in deferred:
                t()
            deferred.clear()
        run_pass(ph0, kts, ph0["io"]["out"][pi], pi)

    if nxt is not None:
        fq.drain(("kv", str(nxt["b"])))
        for t in deferred:
            t()
        deferred.clear()
        for kts in nxt["passes"]:
            run_pass(nxt, kts, nxt["io"]["out"][0], 0)
    for t in deferred:
        t()


def _build_program(phases):
    nc = bacc.Bacc("TRN2", target_bir_lowering=False, debug=False,
                   num_devices=N_CORES)
    for ph in phases:
        s = str(ph["b"])
        Qp, Kp, NK = ph["Qp"], ph["Kp"], ph["NK"]
        npass = 2 if (ph.get("first") and NK > 4) else 1
        io = {
            "xq": nc.dram_tensor("xq" + s, [128, KCH, Qp], F16, kind="ExternalInput"),
            "xk": nc.dram_tensor("xk" + s, [128, KCH, Kp], F16, kind="ExternalInput"),
            "xv": nc.dram_tensor("xv" + s, [128, KCH, Kp], F16, kind="ExternalInput"),
            "out": [nc.dram_tensor(f"out{s}_{p}", [65, 2, Qp], F32,
                                   kind="ExternalOutput") for p in range(npass)],
        }
        ph["io"] = io

    with tile.TileContext(nc) as tc, ExitStack() as ctx:
        P = {
            "w": ctx.enter_context(tc.tile_pool(name="w", bufs=1)),
            "x": ctx.enter_context(tc.tile_pool(name="x", bufs=3)),
            "e": ctx.enter_context(tc.tile_pool(name="e", bufs=8)),
            "ob": ctx.enter_context(tc.tile_pool(name="ob", bufs=10)),
            "persist": ctx.enter_context(tc.tile_pool(name="persist", bufs=1)),
            # 3 sp slots of 2 PSUM banks: two in-flight S pairs + one open
            # filler projection accumulator; ot = 2 banks of PV accumulators
            "sp": ctx.enter_context(tc.tile_pool(name="sp", bufs=3, space="PSUM")),
            "ot": ctx.enter_context(tc.tile_pool(name="ot", bufs=2, space="PSUM")),
        }
        # prime the ACT exp table while the first DMAs are in flight
        warm = P["w"].tile([1, 1], F32, tag="actwarm", name="actwarm")
        nc.vector.memset(warm[:], 0.0)
        nc.scalar.activation(warm[:], warm[:], EXP)
        # dummy matmul stream: ~4us of sustained PE activity releases the
        # HAM clock throttle (1.2 -> 2.4 GHz) while the first input DMAs are
        # still in flight, so the real matmuls start warm. wfill() re-issues
        # short bursts into the (lead-in-idle) ot banks to plug DMA waits.
        wu = P["w"].tile([128, 128], F16, tag="wu", name="wu")
        nc.vector.memset(wu[:], 0.0)
        P["wu"] = wu

        def wfill(n):
            wups = P["ot"].tile([128, 128], F32, tag="ot", name="wups")
            for _ in range(n):
                nc.tensor.matmul(wups[:], lhsT=wu[:], rhs=wu[:],
                                 start=True, stop=True, skip_group_check=True)
        P["wfill"] = wfill
        wfill(44)
        # all three weights in one packed DMA, issued from the (idle) ACT
        # engine so it overlaps the sync queue's first xk issue
        wpd = nc.dram_tensor("wpk", [128, 3, KCH, 128], F16, kind="ExternalInput")
        wpt = P["w"].tile([128, 3, KCH, 128], F16, tag="wpk", name="wpk")
        nc.scalar.dma_start(wpt[:], wpd[:])
        P["wts"] = {"wk": wpt[:, 0], "wv": wpt[:, 1], "wq": wpt[:, 2]}
        _emit_all(nc, tc, P, phases)
    nc.compile()
    return nc


def _prep_xT(X, P):
    """[T, D] -> [128, KCH, P] with x[p, k, t] = X[t, k*128 + p]."""
    Xp = np.ascontiguousarray(X[:P].T)                 # [D, P]
    return np.ascontiguousarray(
        Xp.reshape(KCH, 128, P).transpose(1, 0, 2)).astype(XNP)  # [128, KCH, P]


def _prep_w(W, c):
    """[D, H*DH] -> per-core [128, KCH, 128] slice of heads (2c, 2c+1)."""
    Ws = W[:, c * 128:(c + 1) * 128]                   # [D, 128]
    return np.ascontiguousarray(
        Ws.reshape(KCH, 128, 128).transpose(1, 0, 2)).astype(XNP)


def kernel(Q_seq, K_seq, V_seq, Q_len, V_len, WQ, WK, WV):
    global LAST_EXEC_NS
    Q_seq = np.asarray(Q_seq, dtype=np.float32)
    K_seq = np.asarray(K_seq, dtype=np.float32)
    V_seq = np.asarray(V_seq, dtype=np.float32)
    WQ = np.asarray(WQ, dtype=np.float32)
    WK = np.asarray(WK, dtype=np.float32)
    WV = np.asarray(WV, dtype=np.float32)
    qlen = [int(np.asarray(Q_len)[b, 0]) for b in range(B)]
    vlen = [int(np.asarray(V_len)[b, 0]) for b in range(B)]

    phases = []
    for b in range(B):
        Qp = _ceil_div(qlen[b], 32) * 32   # q only needs 32-elem alignment
        if Qp == 0:
            continue  # whole batch output is zero
        if vlen[b] > 0:
            NK, scale = _ceil_div(vlen[b], 128), SCALE
            vrem = vlen[b] - (NK - 1) * 128
            if vrem == 128:
                vrem = None
        else:
            # all keys masked -> reference softmax degenerates to uniform
            # over all T keys; exp(0*S) = 1 reproduces it exactly.
            NK, scale, vrem = T // 128, 0.0, None
        phases.append(dict(b=b, NK=NK, Qp=Qp, Kp=NK * 128, scale=scale,
                           vrem=vrem, first=not phases))

    out = np.zeros((B, T, H * DH), dtype=np.float32)
    if not phases:
        return out

    nc = _build_program(phases)

    # per-phase data shared by all cores
    shared = {}
    for ph in phases:
        b, s, Kp = ph["b"], str(ph["b"]), ph["Kp"]
        Vb = V_seq[b]
        if 0 < vlen[b] < Kp:
            Vb = Vb.copy()
            Vb[vlen[b]:Kp] = 0.0   # padded keys: zero v rows -> no output term
        shared[s] = {
            "xq" + s: _prep_xT(Q_seq[b], ph["Qp"]),
            "xk" + s: _prep_xT(K_seq[b], Kp),
            "xv" + s: _prep_xT(Vb, Kp),
        }

    in_maps = []
    for c in range(N_CORES):
        m = {}
        for ph in phases:
            m.update(shared[str(ph["b"])])
        m["wpk"] = np.ascontiguousarray(np.stack(
            [_prep_w(WK, c), _prep_w(WV, c), _prep_w(WQ, c)], axis=1))
        in_maps.append(m)

    trace = bool(os.environ.get("BASS_TRACE"))
    if trace:
        _ensure_ntff_hook()
    res = run_bass_kernel_spmd(nc, in_maps, list(range(N_CORES)), trace=trace)
    LAST_EXEC_NS = res.exec_time_ns

    for c in range(N_CORES):
        r = res.results[c]
        for ph in phases:
            b, s = ph["b"], str(ph["b"])
            ql = qlen[b]
            acc = r["out" + s + "_0"].astype(np.float64)
            for p in range(1, ph["npass"]):
                acc += r[f"out{s}_{p}"]
            for h in (0, 1):
                head = 2 * c + h
                num = acc[0:64, h, :ql]
                den = acc[64, h, :ql]
                out[b, :ql, head * DH:(head + 1) * DH] = (num / den).T
    return out


# revision 77
# speedup vs baseline: 1.0145x; 1.0000x over previous
"""Trainium2 Bass kernel: masked multi-head attention, sharded across 8 NeuronCores.

Problem shapes (hardcoded): B=2, T=2048, D=1024, H=16 heads, dh=64.

Sharding: one SPMD program with two phases (one per batch element). In each
phase every core handles 2 of the 16 heads (core c -> heads 2c, 2c+1), so the
16 heads of each batch are spread over all 8 cores. This load-balances the
data-dependent work (Q_len/V_len trim the q/k tile counts per batch).

All matmul operands are fp16 (inputs cast on host): fp32 matmuls cost 4
cycles/row on the TRN2 PE vs 1 for fp16, and fp16 halves the input DMA bytes.
PSUM accumulation stays fp32, so the error vs the fp32 reference is ~1e-3.

Schedule: the first phase's attention is split into kv-chunk PASSES (kt 0..3,
then kt 4..NK-1), each accumulating a partial softmax numerator + denominator
(host sums the partials), so attention starts after only the first 512-key
chunk of K/V lands. Attention is emitted as a FLAT pipeline of (q-chunk,
kt-pair) units -- S matmuls + one exp per head per unit, skewed one unit
ahead of the PV matmuls -- and every later projection (q chunks 1.., later
k/v chunks, and the whole second phase's projections) is chopped into
single-matmul FILLER thunks drained between units: exp on the ACT engine
paces the units (~2us per unit vs ~1.1us of PE work), and the fillers soak
up the PE slack. PSUM: 3 "sp" slots of 2 banks (S pairs + one open filler
projection accumulator) + 2 "ot" banks (PV accumulators). A dummy matmul
stream at t~7us releases the HAM clock throttle before the real work.
Output DMAs are deferred behind later input dma_starts (in-order sync queue).
v_aug rows for tokens >= V_len are zero (host zeroes V) and their ones-column
entry is zeroed on device, replacing the additive -1e12 key mask exactly.
The host does the final divide-by-denominator, query mask and transpose.
"""

import math
import os
from contextlib import ExitStack

import numpy as np

import concourse.bacc as bacc
import concourse.mybir as mybir
import concourse.tile as tile
from concourse.bass_utils import run_bass_kernel_spmd

F32 = mybir.dt.float32
F16 = mybir.dt.float16
EXP = mybir.ActivationFunctionType.Exp
XNP = np.float16

B, T, D, H, DH = 2, 2048, 1024, 16, 64
N_CORES = 8
KCH = D // 128          # 8 contraction chunks of the model dim
SCALE = 1.0 / math.sqrt(DH)
FILL_PER_UNIT = 5       # filler thunks drained after each attention unit

LAST_EXEC_NS = None     # filled when BASS_TRACE=1


def _ensure_ntff_hook():
    """run_bass_kernel_spmd(trace=True) imports antenv.axon_hooks, which some
    containers lack; synthesize it (backed by libaxon_pjrt's NRT profiling)
    so tracing degrades gracefully instead of crashing."""
    import sys
    import types
    try:
        import antenv.axon_hooks  # noqa: F401
        return
    except ImportError:
        pass
    try:
        import antenv
        from trn_agent_boot.trn_boot import _ntff_profile_via_ctypes
        hook = _ntff_profile_via_ctypes("/opt/axon/libaxon_pjrt.so")
    except Exception:
        antenv = None
        hook = None
    try:
        m = types.ModuleType("antenv.axon_hooks")
        m._hook = hook
        m.set_axon_ntff_profile_hook = lambda h: setattr(m, "_hook", h)
        m.get_axon_ntff_profile_hook = lambda: m._hook
        sys.modules["antenv.axon_hooks"] = m
        if antenv is not None:
            antenv.axon_hooks = m
    except Exception:
        pass


def _ceil_div(a, b):
    return -(-a // b)


def _chunk_widths(Qp, first=False):
    """Balanced q-chunk widths (multiples of 32, <=512) -- avoids a
    degenerate tail chunk whose tiny matmuls/exps are all overhead. The
    first phase gets a narrow 256-wide first chunk so attention starts on
    less lead-in DMA."""
    NQC = _ceil_div(Qp, 512)
    base = (Qp // NQC) // 32 * 32
    rem = (Qp - base * NQC) // 32
    return [base + 32 * (i < rem) for i in range(NQC)]


class _FillQueue:
    """Deadline-ordered queue of single-instruction emitter thunks."""

    def __init__(self):
        self.q = []
        self.cnt = {}

    def add(self, tag, thunks):
        for t in thunks:
            self.q.append((tag, t))
        self.cnt[tag] = self.cnt.get(tag, 0) + len(thunks)

    def pop(self, k):
        while k > 0 and self.q:
            tag, t = self.q.pop(0)
            self.cnt[tag] -= 1
            t()
            k -= 1

    def drain(self, tag):
        while self.cnt.get(tag, 0) > 0:
            tg, t = self.q.pop(0)
            self.cnt[tg] -= 1
            t()


def _emit_all(nc, tc, P, phases):
    wts = P["wts"]
    fq = _FillQueue()
    deferred = []   # out-DMA thunks, flushed behind later input dma_starts

    for ph in phases:
        s = str(ph["b"])
        ph["ws"] = _chunk_widths(ph["Qp"], ph.get("first"))
        ph["cst"] = [sum(ph["ws"][:i]) for i in range(len(ph["ws"]))]
        Kp = ph["Kp"]
        kw = []
        r = Kp
        while r > 0:
            kw.append(min(512, r))
            r -= kw[-1]
        ph["kw"] = kw
        ph["kst"] = [sum(kw[:i]) for i in range(len(kw))]
        ph["kmap"] = []
        for ci, w in enumerate(kw):
            for off in range(w // 128):
                ph["kmap"].append((ci, off))
        ph["kcs"] = [None] * len(kw)
        ph["vas"] = [None] * ph["NK"]
        ph["qcs"] = {}
        kts_all = list(range(ph["NK"]))
        na = kw[0] // 128
        ph["passes"] = ([kts_all[:na], kts_all[na:]]
                        if ph.get("first") and ph["NK"] > na else [kts_all])
        ph["npass"] = len(ph["passes"])

    def kproj_thunks(ph, c):
        """DMA issues now; returns single-matmul thunks + the kT copy."""
        s, io = str(ph["b"]), ph["io"]
        n, k0 = ph["kw"][c], ph["kst"][c]
        xt = P["x"].tile([128, KCH, n], F16, tag="xt", name="xt", bufs=3)
        kc = P["persist"].tile([128, n], F16, tag="kT" + s, name="kT" + s,
                               bufs=len(ph["kcs"]))
        ph["kcs"][c] = kc
        box = {}   # psum tile allocated at emission position, not build time
        if ph.get("first") and c == 0:
            # quarters: one dma_start's descriptors fan out over only a few
            # DMA queues (~55GB/s), so parallel splits speed up the lead-in
            for q in range(4):
                nc.sync.dma_start(xt[:, 2 * q:2 * q + 2, :],
                                  io["xk"][:, 2 * q:2 * q + 2, 0:n])
        else:
            # one dma_start per chunk elsewhere: each costs ~600ns of
            # serialized sync-sequencer issue + ~1us of descriptor gen
            nc.sync.dma_start(xt[:], io["xk"][:, :, k0:k0 + n])

        def mk(k):
            def go():
                if k == 0:
                    box["ps"] = P["sp"].tile([128, n], F32, tag="sp", name="kps")
                nc.tensor.matmul(
                    box["ps"][:], lhsT=wts["wk"][:, k, :], rhs=xt[:, k, :],
                    start=(k == 0), stop=(k == KCH - 1), skip_group_check=True)
            return go
        return [mk(k) for k in range(KCH)] + [
            lambda: nc.vector.tensor_copy(kc[:], box["ps"][:])]

    def vproj_thunks(ph, c):
        s, io = str(ph["b"]), ph["io"]
        NK, vrem = ph["NK"], ph["vrem"]
        n, k0 = ph["kw"][c], ph["kst"][c]
        ng = n // 128
        xtv = P["x"].tile([128, KCH, n], F16, tag="xtv", name="xtv", bufs=3)
        if ph.get("first") and c == 0:
            # lead-in: one half on the (idle) ACT engine's DGE in parallel
            # with the sync queue's xk quarters, the other behind xk on sync
            nc.scalar.dma_start(xtv[:, 0:4, :], io["xv"][:, 0:4, 0:n])
            nc.sync.dma_start(xtv[:, 4:8, :], io["xv"][:, 4:8, 0:n])
        else:
            nc.sync.dma_start(xtv[:], io["xv"][:, :, k0:k0 + n])
        thunks = []
        for j in range(ng):
            kt = k0 // 128 + j
            # 128 weight columns (65 used) so the PV matmul's LDWEIGHTS gets
            # fast-weight-load and hides under the previous matmul
            va = P["persist"].tile([128, 2, 128], F16, tag="va" + s,
                                   name="va" + s, bufs=NK)
            if kt == NK - 1 and vrem is not None:
                # partial last key tile: ones only on the valid rows, so
                # padded keys add nothing to the softmax denominator
                nc.vector.memset(va[:, :, 64:128], 0.0)
                nc.vector.memset(va[0:vrem, :, 64:65], 1.0)
            else:
                nc.vector.memset(va[:, :, 64:128], 1.0)
            ph["vas"][kt] = va
            box = {}

            def mk(k, j=j, box=box):
                def go():
                    if k == 0:
                        box["ps"] = P["sp"].tile([128, 128], F32, tag="sp",
                                                 name="ps2")
                    nc.tensor.matmul(
                        box["ps"][:], lhsT=xtv[:, k, j * 128:(j + 1) * 128],
                        rhs=wts["wv"][:, k, :],
                        start=(k == 0), stop=(k == KCH - 1),
                        skip_group_check=True)
                return go
            thunks += [mk(k) for k in range(KCH)]
            thunks.append(lambda va=va, box=box: nc.vector.tensor_copy(
                va[:, :, 0:64], box["ps"][:].rearrange("p (g d) -> p g d", g=2)))
        return thunks

    def qproj_thunks(ph, c):
        s, io = str(ph["b"]), ph["io"]
        n, c0 = ph["ws"][c], ph["cst"][c]
        xtq = P["x"].tile([128, KCH, n], F16, tag="xtq", name="xtq", bufs=4)
        if ph.get("first") and c == 0:
            nc.scalar.dma_start(xtq[:, 0:4, :], io["xq"][:, 0:4, c0:c0 + n])
            nc.sync.dma_start(xtq[:, 4:8, :], io["xq"][:, 4:8, c0:c0 + n])
        else:
            nc.sync.dma_start(xtq[:], io["xq"][:, :, c0:c0 + n])
        qc = P["persist"].tile([128, n], F16, tag="qT" + s, name="qT" + s,
                               bufs=len(ph["ws"]))
        ph["qcs"][c] = qc
        box = {}

        def mk(k):
            def go():
                if k == 0:
                    box["ps"] = P["sp"].tile([128, n], F32, tag="sp", name="qps")
                nc.tensor.matmul(
                    box["ps"][:], lhsT=wts["wq"][:, k, :], rhs=xtq[:, k, :],
                    start=(k == 0), stop=(k == KCH - 1), skip_group_check=True)
            return go
        return [mk(k) for k in range(KCH)] + [
            lambda: nc.vector.tensor_copy(qc[:], box["ps"][:])]

    def run_pass(ph, kts, out_d, pi):
        """Flat skew-1 pipeline over (chunk, kt-pair) units with fillers."""
        s, scale = str(ph["b"]), ph["scale"]
        # one SBUF staging buffer for the whole pass -> a single out-DMA
        obp = P["ob"].tile([65, 2, ph["Qp"]], F32, tag=f"ob{s}_{pi}",
                           name="obp", bufs=1)
        kcs, vas, qcs = ph["kcs"], ph["vas"], ph["qcs"]
        units = []
        groups = [kts[j:j + 2] for j in range(0, len(kts), 2)]
        for c in range(len(ph["ws"])):
            for gi, g in enumerate(groups):
                units.append((c, g, gi == 0, gi == len(groups) - 1))
        otds = {}

        def emit_sg(u):
            c, g, first, last = u
            n = ph["ws"][c]
            qc = qcs[c]
            es = []
            for h in (0, 1):
                # pad the pair stride to a full 2KB PSUM bank: a matmul
                # output slice must not straddle a bank boundary
                sps = P["sp"].tile([128, len(g), n], F32, tag="sp", name="sps",
                                   padded_shape=[128, len(g), 512])
                for i, kt in enumerate(g):
                    ci, off = ph["kmap"][kt]
                    nc.tensor.matmul(
                        sps[:, i, :],
                        lhsT=kcs[ci][h * 64:(h + 1) * 64,
                                     off * 128:off * 128 + 128],
                        rhs=qc[h * 64:(h + 1) * 64, :],
                        start=True, stop=True)
                e = P["e"].tile([128, len(g), n], F16, tag="e", name="e")
                nc.scalar.activation(e[:], sps[:], EXP, scale=scale)
                es.append(e)
            return es

        def emit_pv(u, es):
            c, g, first, last = u
            n, c0 = ph["ws"][c], ph["cst"][c]
            if first:
                otds[c] = [P["ot"].tile([128, n], F32, tag="ot", name="otd")
                           for _ in (0, 1)]
            for h in (0, 1):
                for i, kt in enumerate(g):
                    nc.tensor.matmul(otds[c][h][:], lhsT=vas[kt][:, h, :],
                                     rhs=es[h][:, i, :],
                                     start=(kt == kts[0]), stop=(kt == kts[-1]),
                                     skip_group_check=True)
            if last:
                for h in (0, 1):
                    nc.vector.tensor_copy(obp[:, h, c0:c0 + n],
                                          otds[c][h][0:65, :])

        prev = None
        for u in units:
            if u[2]:  # first unit of chunk c: its qc must be projected
                fq.drain(("q", s, u[0]))
            es = emit_sg(u)
            if prev is not None:
                emit_pv(*prev)
            prev = (u, es)
            fq.pop(FILL_PER_UNIT)
        emit_pv(*prev)
        deferred.append(lambda: nc.sync.dma_start(out_d[:], obp[:]))

    # ---- phase 0 lead-in: first kv chunk + first q chunk, emitted directly
    # with dummy-matmul bursts plugging the DMA-paced PE waits
    ph0 = phases[0]
    kth = kproj_thunks(ph0, 0)
    vth = vproj_thunks(ph0, 0)
    qth = qproj_thunks(ph0, 0)
    for t in kth[:4]:
        t()
    P["wfill"](6)
    for t in kth[4:]:
        t()
    P["wfill"](6)
    for t in vth:
        t()
    P["wfill"](6)
    for t in qth:
        t()
    nxt = phases[1] if len(phases) > 1 else None

    for pi, kts in enumerate(ph0["passes"]):
        last_pass = pi == len(ph0["passes"]) - 1
        s0 = str(ph0["b"])
        if pi == 0:
            # queue later q chunks (deadline: their attention chunk), later
            # kv chunks (deadline: pass B), and the whole second phase's
            # projections (deadline: its attention) as fillers -- all their
            # input DMAs issue NOW, long before the matmuls need them
            for c in range(1, len(ph0["ws"])):
                fq.add(("q", s0, c), qproj_thunks(ph0, c))
            for c in range(1, len(ph0["kcs"])):
                fq.add(("kv", s0), kproj_thunks(ph0, c) + vproj_thunks(ph0, c))
            if nxt is not None:
                s1 = str(nxt["b"])
                for c in range(len(nxt["kcs"])):
                    fq.add(("kv", s1),
                           kproj_thunks(nxt, c) + vproj_thunks(nxt, c))
                for c in range(len(nxt["ws"])):
                    fq.add(("q", s1, c), qproj_thunks(nxt, c))
        else:
            fq.drain(("kv", s0))
            for t in deferred:
                t()
            deferred.clear()
        run_pass(ph0, kts, ph0["io"]["out"][pi], pi)

    if nxt is not None:
        fq.drain(("kv", str(nxt["b"])))
        for t in deferred:
            t()
        deferred.clear()
        for kts in nxt["passes"]:
            run_pass(nxt, kts, nxt["io"]["out"][0], 0)
    for t in deferred:
        t()


def _build_program(phases):
    nc = bacc.Bacc("TRN2", target_bir_lowering=False, debug=False,
                   num_devices=N_CORES)
    for ph in phases:
        s = str(ph["b"])
        Qp, Kp, NK = ph["Qp"], ph["Kp"], ph["NK"]
        npass = 2 if (ph.get("first") and NK > 4) else 1
        io = {
            "xq": nc.dram_tensor("xq" + s, [128, KCH, Qp], F16, kind="ExternalInput"),
            "xk": nc.dram_tensor("xk" + s, [128, KCH, Kp], F16, kind="ExternalInput"),
            "xv": nc.dram_tensor("xv" + s, [128, KCH, Kp], F16, kind="ExternalInput"),
            "out": [nc.dram_tensor(f"out{s}_{p}", [65, 2, Qp], F32,
                                   kind="ExternalOutput") for p in range(npass)],
        }
        ph["io"] = io

    with tile.TileContext(nc) as tc, ExitStack() as ctx:
        P = {
            "w": ctx.enter_context(tc.tile_pool(name="w", bufs=1)),
            "x": ctx.enter_context(tc.tile_pool(name="x", bufs=3)),
            "e": ctx.enter_context(tc.tile_pool(name="e", bufs=6)),
            "ob": ctx.enter_context(tc.tile_pool(name="ob", bufs=10)),
            "persist": ctx.enter_context(tc.tile_pool(name="persist", bufs=1)),
            # 3 sp slots of 2 PSUM banks: two in-flight S pairs + one open
            # filler projection accumulator; ot = 2 banks of PV accumulators
            "sp": ctx.enter_context(tc.tile_pool(name="sp", bufs=3, space="PSUM")),
            "ot": ctx.enter_context(tc.tile_pool(name="ot", bufs=2, space="PSUM")),
        }
        # prime the ACT exp table while the first DMAs are in flight
        warm = P["w"].tile([1, 1], F32, tag="actwarm", name="actwarm")
        nc.vector.memset(warm[:], 0.0)
        nc.scalar.activation(warm[:], warm[:], EXP)
        # dummy matmul stream: ~4us of sustained PE activity releases the
        # HAM clock throttle (1.2 -> 2.4 GHz) while the first input DMAs are
        # still in flight, so the real matmuls start warm. wfill() re-issues
        # short bursts into the (lead-in-idle) ot banks to plug DMA waits.
        wu = P["w"].tile([128, 128], F16, tag="wu", name="wu")
        nc.vector.memset(wu[:], 0.0)
        P["wu"] = wu

        def wfill(n):
            wups = P["ot"].tile([128, 128], F32, tag="ot", name="wups")
            for _ in range(n):
                nc.tensor.matmul(wups[:], lhsT=wu[:], rhs=wu[:],
                                 start=True, stop=True, skip_group_check=True)
        P["wfill"] = wfill
        wfill(44)
        # all three weights in one packed DMA, issued from the (idle) ACT
        # engine so it overlaps the sync queue's first xk issue
        wpd = nc.dram_tensor("wpk", [128, 3, KCH, 128], F16, kind="ExternalInput")
        wpt = P["w"].tile([128, 3, KCH, 128], F16, tag="wpk", name="wpk")
        nc.scalar.dma_start(wpt[:], wpd[:])
        P["wts"] = {"wk": wpt[:, 0], "wv": wpt[:, 1], "wq": wpt[:, 2]}
        _emit_all(nc, tc, P, phases)
    nc.compile()
    return nc


def _prep_xT(X, P):
    """[T, D] -> [128, KCH, P] with x[p, k, t] = X[t, k*128 + p]."""
    Xp = np.ascontiguousarray(X[:P].T)                 # [D, P]
    return np.ascontiguousarray(
        Xp.reshape(KCH, 128, P).transpose(1, 0, 2)).astype(XNP)  # [128, KCH, P]


def _prep_w(W, c):
    """[D, H*DH] -> per-core [128, KCH, 128] slice of heads (2c, 2c+1)."""
    Ws = W[:, c * 128:(c + 1) * 128]                   # [D, 128]
    return np.ascontiguousarray(
        Ws.reshape(KCH, 128, 128).transpose(1, 0, 2)).astype(XNP)


def kernel(Q_seq, K_seq, V_seq, Q_len, V_len, WQ, WK, WV):
    global LAST_EXEC_NS
    Q_seq = np.asarray(Q_seq, dtype=np.float32)
    K_seq = np.asarray(K_seq, dtype=np.float32)
    V_seq = np.asarray(V_seq, dtype=np.float32)
    WQ = np.asarray(WQ, dtype=np.float32)
    WK = np.asarray(WK, dtype=np.float32)
    WV = np.asarray(WV, dtype=np.float32)
    qlen = [int(np.asarray(Q_len)[b, 0]) for b in range(B)]
    vlen = [int(np.asarray(V_len)[b, 0]) for b in range(B)]

    phases = []
    for b in range(B):
        Qp = _ceil_div(qlen[b], 32) * 32   # q only needs 32-elem alignment
        if Qp == 0:
            continue  # whole batch output is zero
        if vlen[b] > 0:
            NK, scale = _ceil_div(vlen[b], 128), SCALE
            vrem = vlen[b] - (NK - 1) * 128
            if vrem == 128:
                vrem = None
        else:
            # all keys masked -> reference softmax degenerates to uniform
            # over all T keys; exp(0*S) = 1 reproduces it exactly.
            NK, scale, vrem = T // 128, 0.0, None
        phases.append(dict(b=b, NK=NK, Qp=Qp, Kp=NK * 128, scale=scale,
                           vrem=vrem, first=not phases))

    out = np.zeros((B, T, H * DH), dtype=np.float32)
    if not phases:
        return out

    nc = _build_program(phases)

    # per-phase data shared by all cores
    shared = {}
    for ph in phases:
        b, s, Kp = ph["b"], str(ph["b"]), ph["Kp"]
        Vb = V_seq[b]
        if 0 < vlen[b] < Kp:
            Vb = Vb.copy()
            Vb[vlen[b]:Kp] = 0.0   # padded keys: zero v rows -> no output term
        shared[s] = {
            "xq" + s: _prep_xT(Q_seq[b], ph["Qp"]),
            "xk" + s: _prep_xT(K_seq[b], Kp),
            "xv" + s: _prep_xT(Vb, Kp),
        }

    in_maps = []
    for c in range(N_CORES):
        m = {}
        for ph in phases:
            m.update(shared[str(ph["b"])])
        m["wpk"] = np.ascontiguousarray(np.stack(
            [_prep_w(WK, c), _prep_w(WV, c), _prep_w(WQ, c)], axis=1))
        in_maps.append(m)

    trace = bool(os.environ.get("BASS_TRACE"))
    if trace:
        _ensure_ntff_hook()
    res = run_bass_kernel_spmd(nc, in_maps, list(range(N_CORES)), trace=trace)
    LAST_EXEC_NS = res.exec_time_ns

    for c in range(N_CORES):
        r = res.results[c]
        for ph in phases:
            b, s = ph["b"], str(ph["b"])
            ql = qlen[b]
            acc = r["out" + s + "_0"].astype(np.float64)
            for p in range(1, ph["npass"]):
                acc += r[f"out{s}_{p}"]
            for h in (0, 1):
                head = 2 * c + h
                num = acc[0:64, h, :ql]
                den = acc[64, h, :ql]
                out[b, :ql, head * DH:(head + 1) * DH] = (num / den).T
    return out
